# revision 3
# baseline (speedup 1.0000x reference)
"""EvolveGCN layer on 8 trn2 NeuronCores.

Math: out = relu(segment_sum(h[src] * ew, dst) @ W)   (projection commutes
with the linear aggregation, so we aggregate raw h first and run one GEMM
per 128-dst block afterwards -- no inter-core communication at all).

Sharding: dst nodes are range-partitioned across the 8 cores (12500 each).
Each core gets the full h (gathered from its own DRAM), its edge partition
(sorted by (dst_block, src)), aggregates per 128-dst block via one-hot
scatter matmuls in PSUM, transposes, multiplies by W, applies ReLU.

Gather: dma_gather (Q7 mlp library) with int16 indices wrapped in 16
partitions; h is split into 4 row groups of 25000 so indices fit int16.
"""
import os
import sys

sys.path.insert(0, "/opt/trn_rl_repo")
sys.path.insert(0, "/opt/trn_rl_repo/concourse")

import numpy as np

N_NODES = 100000
N_CORES = 8
D = 512
P = 128
SHARD = N_NODES // N_CORES          # 12500 dst nodes per core
NBLK = (SHARD + P - 1) // P         # 98 dst blocks per core
GROUP = 25000                       # src rows per dma_gather group
NGRP = N_NODES // GROUP             # 4

_LAST_RUN = {}                      # test.py reads exec_time_ns from here


def _host_prep(src, dst, edge_weight):
    """Partition/sort edges; build per-core meta planes.

    Returns dict with per-core arrays and the shared chunk structure.
    """
    src = np.asarray(src).astype(np.int64)
    dst = np.asarray(dst).astype(np.int64)
    ew = np.asarray(edge_weight).astype(np.float32)

    core = dst // SHARD
    per_core = []
    for k in range(N_CORES):
        m = core == k
        s, d, w = src[m], dst[m] - k * SHARD, ew[m]
        blk = d // P
        g = s // GROUP
        order = np.lexsort((s, g, blk))       # sort by (block, group, src)
        s, d, w, blk, g = s[order], d[order], w[order], blk[order], g[order]
        # counts[b, gi] = edges of core k in (block b, group gi)
        counts = np.zeros((NBLK, NGRP), dtype=np.int64)
        np.add.at(counts, (blk, g), 1)
        per_core.append(dict(s=s, d=d, w=w, counts=counts))

    all_counts = np.stack([pc["counts"] for pc in per_core])  # [8, NBLK, NGRP]
    maxc = all_counts.max(axis=0)                             # [NBLK, NGRP]
    cbg = -(-maxc // P)                                       # chunks per (b, g)
    cb = cbg.sum(axis=1)                                      # chunks per block
    tc = int(cb.sum())                                        # total chunks

    # chunk-column base per (b, g)
    base = np.zeros((NBLK, NGRP), dtype=np.int64)
    run = 0
    for b in range(NBLK):
        for gi in range(NGRP):
            base[b, gi] = run
            run += cbg[b, gi]

    metas = []
    for k in range(N_CORES):
        pc = per_core[k]
        dl_plane = np.zeros((P, tc), dtype=np.float32)
        w_plane = np.zeros((P, tc), dtype=np.float32)
        idx_flat = np.full((tc * P,), -1, dtype=np.int16)  # slot-major edge idx
        pos = 0
        for b in range(NBLK):
            for gi in range(NGRP):
                n = int(pc["counts"][b, gi])
                nslot = int(cbg[b, gi]) * P
                if nslot == 0:
                    continue
                sl = slice(pos, pos + n)
                i = np.arange(n)
                c0 = int(base[b, gi])
                lanes = i % P
                cols = c0 + i // P
                dl_plane[lanes, cols] = (pc["d"][sl] - b * P).astype(np.float32)
                w_plane[lanes, cols] = pc["w"][sl]
                rel = (pc["s"][sl] - gi * GROUP).astype(np.int16)
                mc = int(maxc[b, gi])
                seg = np.full((nslot,), -1, dtype=np.int16)
                seg[:n] = rel
                seg[n:mc] = 0                  # pad-to-max gathers row 0
                idx_flat[c0 * P:c0 * P + nslot] = seg
                pos += n
        # wrap idxs: position i -> [i % 16, i // 16], replicated to 128 parts
        wrapped = np.zeros((16, tc * P // 16), dtype=np.int16)
        ii = np.arange(tc * P)
        wrapped[ii % 16, ii // 16] = idx_flat
        idx_plane = np.tile(wrapped, (8, 1))
        metas.append(dict(dl=dl_plane, w=w_plane, idx=idx_plane))

    return dict(metas=metas, maxc=maxc, cbg=cbg, cb=cb, base=base, tc=tc)


def _build_program(prep, mm_f32r=True):
    import concourse.bass as bass
    import concourse.mybir as mybir
    import concourse.tile as tile
    from concourse import bacc

    maxc, cbg, cb, base, tc = (
        prep["maxc"], prep["cbg"], prep["cb"], prep["base"], prep["tc"],
    )
    cbmax = int(cb.max())
    f32 = mybir.dt.float32
    f32r = mybir.dt.float32r

    nc = bacc.Bacc(None, target_bir_lowering=False, debug=True)
    mmdt = f32r if mm_f32r else f32
    h_t = nc.declare_dram_parameter("h", [N_NODES, D], mmdt, isOutput=False)
    w_t = nc.declare_dram_parameter("wmat", [D, D], mmdt, isOutput=False)
    dl_t = nc.declare_dram_parameter("dl", [P, tc], f32, isOutput=False)
    ww_t = nc.declare_dram_parameter("ww", [P, tc], f32, isOutput=False)
    ix_t = nc.declare_dram_parameter("ix", [P, tc * 8], mybir.dt.int16, isOutput=False)
    io_t = nc.declare_dram_parameter("iota", [P, P], f32, isOutput=False)
    id_t = nc.declare_dram_parameter("ident", [P, P], f32, isOutput=False)
    out_t = nc.declare_dram_parameter("out", [NBLK * P, D], f32, isOutput=True)


    with tile.TileContext(nc) as tcx:
        with (
            tcx.tile_pool(name="const", bufs=1) as cpool,
            tcx.tile_pool(name="xp", bufs=2) as xp,
            tcx.tile_pool(name="sp", bufs=4) as spool,
            tcx.tile_pool(name="cp", bufs=2) as copies,
            tcx.tile_pool(name="pp", bufs=2, space="PSUM") as pp,
        ):
            dl_s = cpool.tile([P, tc], f32)
            ww_s = cpool.tile([P, tc], f32)
            ix_s = cpool.tile([P, tc * 8], mybir.dt.int16)
            io_s = cpool.tile([P, P], f32)
            id_s = cpool.tile([P, P], f32)
            wm_s = cpool.tile([P, 4, D], mmdt)   # W[j*128+p, o] at [p, j, o]
            nc.sync.dma_start(out=dl_s[:], in_=dl_t[:])
            nc.sync.dma_start(out=ww_s[:], in_=ww_t[:])
            nc.sync.dma_start(out=ix_s[:], in_=ix_t[:])
            nc.sync.dma_start(out=io_s[:], in_=io_t[:])
            nc.sync.dma_start(out=id_s[:], in_=id_t[:])
            nc.sync.dma_start(
                out=wm_s[:],
                in_=w_t[:].rearrange("(a p) o -> p a o", p=P),
            )

            # zero the X slots once: stale tails are masked by w=0 in S, but
            # must be finite
            for _ in range(2):
                xz = xp.tile([P, cbmax, D], f32, tag="X")
                nc.vector.memset(xz[:], 0)

            for b in range(NBLK):
                cb_b = int(cb[b])
                if cb_b == 0:
                    continue
                X = xp.tile([P, cbmax, D], mmdt, tag="X")
                for gi in range(NGRP):
                    n_ch = int(cbg[b, gi])
                    if n_ch == 0:
                        continue
                    c0 = int(base[b, gi]) - int(base[b, 0])
                    p0 = int(base[b, gi])  # global chunk col in meta planes
                    nc.gpsimd.dma_gather(
                        out_ap=X[:, c0:c0 + n_ch, :],
                        in_ap=h_t[gi * GROUP:(gi + 1) * GROUP, :],
                        idxs_ap=ix_s[:, p0 * 8:(p0 + n_ch) * 8],
                        num_idxs=n_ch * P,
                        num_idxs_reg=int(maxc[b, gi]),
                        elem_size=D,
                        single_packet=False,
                    )
                agg_ps = pp.tile([P, D], f32, space="PSUM", tag="agg")
                gb = int(base[b, 0])
                for c in range(cb_b):
                    S = spool.tile([P, P], mmdt, tag="S")
                    nc.vector.tensor_scalar(
                        out=S[:], in0=io_s[:],
                        scalar1=dl_s[:, gb + c:gb + c + 1],
                        scalar2=ww_s[:, gb + c:gb + c + 1],
                        op0=mybir.AluOpType.is_equal,
                        op1=mybir.AluOpType.mult,
                    )
                    nc.tensor.matmul(
                        out=agg_ps[:], lhsT=S[:], rhs=X[:, c, :],
                        start=(c == 0), stop=(c == cb_b - 1),
                    )
                agg_sb = copies.tile([P, D], f32, tag="aggsb")
                nc.vector.tensor_copy(out=agg_sb[:], in_=agg_ps[:])
                aggT_ps = pp.tile([P, D], f32, space="PSUM", tag="aggT")
                for j in range(4):
                    nc.tensor.transpose(
                        out=aggT_ps[:, j * P:(j + 1) * P],
                        in_=agg_sb[:, j * P:(j + 1) * P],
                        identity=id_s[:],
                    )
                aggT_sb = copies.tile([P, D], mmdt, tag="aggTsb")
                nc.vector.tensor_copy(out=aggT_sb[:], in_=aggT_ps[:])
                out_ps = pp.tile([P, D], f32, space="PSUM", tag="out")
                for j in range(4):
                    nc.tensor.matmul(
                        out=out_ps[:],
                        lhsT=aggT_sb[:, j * P:(j + 1) * P],
                        rhs=wm_s[:, j, :],
                        start=(j == 0), stop=(j == 3),
                    )
                out_sb = copies.tile([P, D], f32, tag="outsb")
                nc.scalar.activation(
                    out_sb[:], out_ps[:], mybir.ActivationFunctionType.Relu
                )
                nc.sync.dma_start(
                    out=out_t[b * P:(b + 1) * P, :], in_=out_sb[:]
                )
    nc.compile()
    return nc


def kernel(h, weight, edge_weight, src, dst):
    from concourse.bass_utils import run_bass_kernel_spmd

    h = np.ascontiguousarray(np.asarray(h), dtype=np.float32)
    weight = np.ascontiguousarray(np.asarray(weight), dtype=np.float32)

    prep = _host_prep(src, dst, edge_weight)
    nc = _build_program(prep, mm_f32r=os.environ.get("KERNEL_FP32", "0") != "1")

    iota = np.broadcast_to(
        np.arange(P, dtype=np.float32)[None, :], (P, P)
    ).copy()
    ident = np.eye(P, dtype=np.float32)
    in_maps = []
    for k in range(N_CORES):
        m = prep["metas"][k]
        in_maps.append({
            "h": h, "wmat": weight, "dl": m["dl"], "ww": m["w"],
            "ix": m["idx"], "iota": iota, "ident": ident,
        })

    trace = os.environ.get("KERNEL_TRACE", "0") == "1"
    kw = {}
    if trace:
        kw = dict(trace=True)
    res = run_bass_kernel_spmd(nc, in_maps, core_ids=list(range(N_CORES)), **kw)
    _LAST_RUN["exec_time_ns"] = res.exec_time_ns
    _LAST_RUN["results"] = res

    out = np.empty((N_NODES, D), dtype=np.float32)
    for k in range(N_CORES):
        out[k * SHARD:(k + 1) * SHARD] = res.results[k]["out"][:SHARD]
    return out
